# revision 4
# baseline (speedup 1.0000x reference)
"""Mamba-1 selective-scan recurrence kernel for Trainium2 (8 NeuronCores).

Problem: B=2, L=2048, D=1024, N=16, R=64 (f32).
  x_dbl = hidden @ W_xproj.T ; dt_low, Bm, Cm = split(x_dbl, [R, N, N])
  delta = softplus(dt_low @ W_dt.T + b_dt)
  h_t   = exp(delta_t*A) * h_{t-1} + (delta_t*x_t) * B_t ;  y_t = sum_n C_t(n) h_t(:,n)

Sharding: core = (batch b in {0,1}) x (channel quarter ds in {0..3}); each core
computes y for its 256 channels of one batch.  No cross-core communication.
The per-core input `x` is hidden[b] with columns permuted so the core's own
channel slice sits at columns [0:256] (the SPMD program is identical on all
cores; the data layout encodes the shard).

Per-core device pipeline (d-on-partitions layout, time on the free axis):
  1. DMA x (2048,1024) -> SBUF, PE-transpose to XT (d,t).
  2. PE: x_dblT (96,2048) = W_xprojT.T @ XT;  split dt_lowT / (B;C) rows.
  3. PE: pre (256,2048) = W_dtT.T @ dt_lowT;
     ACT: deltaT = Ln(Exp(pre + b_dt) + 1) = softplus(pre + b_dt)
     (no Softplus table on ACT, but ln+exp+copy share one table set);
     DVE: uT = deltaT * XT[own 256 channels].
  4. per n in 0..15:  PE selector-matmul broadcasts B/C row n across 128
     partitions into PSUM; ACT: dA = Exp(A[:,n] * deltaT) (fused
     per-partition scale); DVE: dBx = uT*Bb;
     h = tensor_tensor_scan(dA, dBx); y += h*Cb.
  5. PE-transpose y -> (t,d), DMA out.
"""

import sys

for _p in ("/opt/trn_rl_repo",):
    if _p not in sys.path:
        sys.path.insert(0, _p)

import numpy as np

import concourse.bass as bass  # noqa: F401
import concourse.tile as tile
from concourse import bacc, mybir
from concourse.bass_utils import run_bass_kernel_spmd

F32 = mybir.dt.float32

B, L, D, N, R = 2, 2048, 1024, 16, 64
NCORES = 8
DSH = D // 4          # channels per core
P = 128               # partitions
NDT = DSH // P        # d-tiles per core (2)
E = R + 2 * N         # x_dbl feature dim (96)

_CACHE = {}


def build_nc(Lc=L):
    """Build the per-core Bass program (identical across all cores)."""
    nc = bacc.Bacc("TRN2", target_bir_lowering=False, debug=False,
                   num_devices=NCORES)

    x_d = nc.dram_tensor("x", [Lc, D], F32, kind="ExternalInput")
    wxT_d = nc.dram_tensor("wxT", [D, E], F32, kind="ExternalInput")
    wdtT_d = nc.dram_tensor("wdtT", [R, DSH], F32, kind="ExternalInput")
    bdt_d = nc.dram_tensor("bdt", [DSH, 1], F32, kind="ExternalInput")
    acol_d = nc.dram_tensor("acol", [DSH, N], F32, kind="ExternalInput")
    ident_d = nc.dram_tensor("ident", [P, P], F32, kind="ExternalInput")
    selbc_d = nc.dram_tensor("selbc", [2 * N, 2 * N * P], F32,
                             kind="ExternalInput")
    y_d = nc.dram_tensor("y", [Lc, DSH], F32, kind="ExternalOutput")

    with tile.TileContext(nc) as tc:
        _emit(tc, nc, x_d, wxT_d, wdtT_d, bdt_d, acol_d, ident_d, selbc_d,
              y_d, Lc)
    nc.compile()
    return nc


def _emit(tc, nc, x_d, wxT_d, wdtT_d, bdt_d, acol_d, ident_d, selbc_d, y_d,
          Lc):
    mult = mybir.AluOpType.mult
    add = mybir.AluOpType.add
    AF = mybir.ActivationFunctionType

    NT = Lc // P          # time chunks of 128
    NJ = D // P           # full-width d-tiles (8)
    NC512 = Lc // 512
    HLF = Lc // 2

    with (
        tc.tile_pool(name="persist", bufs=1) as persist,
        tc.tile_pool(name="consts", bufs=1) as consts,
    ):
        ident = consts.tile([P, P], F32, tag="ident")
        nc.sync.dma_start(ident[:], ident_d[:])
        acol = consts.tile([P, NDT, N], F32, tag="acol")
        bdt = consts.tile([P, NDT], F32, tag="bdt")
        for dt in range(NDT):
            nc.sync.dma_start(acol[:, dt, :], acol_d[dt * P:(dt + 1) * P, :])
            nc.sync.dma_start(bdt[:, dt:dt + 1], bdt_d[dt * P:(dt + 1) * P, :])
        selbc = consts.tile([2 * N, 2 * N, P], F32, tag="selbc")
        nc.sync.dma_start(selbc[:], selbc_d[:].rearrange(
            "k (q m) -> k q m", q=2 * N))

        # persistent SBUF tensors
        deltaT = persist.tile([P, NDT, Lc], F32, tag="deltaT")
        uT = persist.tile([P, NDT, Lc], F32, tag="uT")
        bc = persist.tile([2 * N, Lc], F32, tag="bc")   # rows 0:N = B, N:2N = C
        dtl = persist.tile([R, Lc], F32, tag="dtl")
        yacc = persist.tile([P, NDT, Lc], F32, tag="yacc")

        # ========== phase 1-3: X load + transpose, projections ==========
        with (
            tc.tile_pool(name="xt", bufs=1) as xt_pool,
            tc.tile_pool(name="xload", bufs=6) as xload,
            tc.tile_pool(name="ps_t", bufs=4, space="PSUM") as ps_t,
            tc.tile_pool(name="ps_mm", bufs=2, space="PSUM") as ps_mm,
            tc.tile_pool(name="wpool", bufs=1) as wpool,
        ):
            XT = xt_pool.tile([P, NJ, Lc], F32, tag="XT")
            for ig in range(NT // 4):
                xis = []
                for k in range(4):
                    i = ig * 4 + k
                    xi = xload.tile([P, D], F32, tag="xi")
                    nc.sync.dma_start(xi[:], x_d[i * P:(i + 1) * P, :])
                    xis.append(xi)
                for j in range(NJ):
                    pt = ps_t.tile([P, 512], F32, tag="pt")
                    for k in range(4):
                        nc.tensor.transpose(pt[:, k * P:(k + 1) * P],
                                            xis[k][:, j * P:(j + 1) * P],
                                            ident[:])
                    nc.scalar.copy(XT[:, j, ig * 512:(ig + 1) * 512], pt[:])

            wx = wpool.tile([P, NJ, E], F32, tag="wx")
            for j in range(NJ):
                nc.sync.dma_start(wx[:, j, :], wxT_d[j * P:(j + 1) * P, :])
            wdt = wpool.tile([R, DSH], F32, tag="wdt")
            nc.sync.dma_start(wdt[:], wdtT_d[:])

            # x_dblT (E, Lc); B/C rows land at partitions 64:96 of PSUM and
            # are copied (partition-shifted) to bc at base 0.
            for c in range(NC512):
                xdbl_ps = ps_mm.tile([P, 512], F32, tag="mm")
                for j in range(NJ):
                    nc.tensor.matmul(xdbl_ps[0:E, :], wx[:, j, :],
                                     XT[:, j, c * 512:(c + 1) * 512],
                                     start=(j == 0), stop=(j == NJ - 1))
                nc.scalar.copy(dtl[:, c * 512:(c + 1) * 512], xdbl_ps[0:R, :])
                nc.scalar.copy(bc[:, c * 512:(c + 1) * 512], xdbl_ps[R:E, :])

            # deltaT = Ln(Exp(pre + b_dt) + 1) = softplus; uT = deltaT * x
            for dt in range(NDT):
                for c in range(NC512):
                    dp = ps_mm.tile([P, 512], F32, tag="mm")
                    nc.tensor.matmul(dp[:], wdt[:, dt * P:(dt + 1) * P],
                                     dtl[:, c * 512:(c + 1) * 512],
                                     start=True, stop=True)
                    nc.scalar.activation(
                        deltaT[:, dt, c * 512:(c + 1) * 512], dp[:],
                        AF.Exp, bias=bdt[:, dt:dt + 1], scale=1.0)
                nc.scalar.activation(deltaT[:, dt, :], deltaT[:, dt, :],
                                     AF.Ln, bias=1.0, scale=1.0)
                nc.vector.tensor_mul(uT[:, dt, :], deltaT[:, dt, :],
                                     XT[:, dt, :])

        # ========== phase 4: per-state recurrence ==========
        with (
            tc.tile_pool(name="bc_ps", bufs=2, space="PSUM") as bc_ps,
            tc.tile_pool(name="work", bufs=4) as work,
            tc.tile_pool(name="chp", bufs=2) as chp,
        ):
            for n in range(N):
                bb = []
                cc = []
                for h in range(2):
                    bbt = bc_ps.tile([P, HLF], F32, tag="bb")
                    cct = bc_ps.tile([P, HLF], F32, tag="cc")
                    for q in range(HLF // 512):
                        col = h * HLF + q * 512
                        nc.tensor.matmul(bbt[:, q * 512:(q + 1) * 512],
                                         selbc[:, n, :],
                                         bc[:, col:col + 512],
                                         start=True, stop=True)
                        nc.tensor.matmul(cct[:, q * 512:(q + 1) * 512],
                                         selbc[:, N + n, :],
                                         bc[:, col:col + 512],
                                         start=True, stop=True)
                    bb.append(bbt)
                    cc.append(cct)
                for dt in range(NDT):
                    da = work.tile([P, Lc], F32, tag="da")
                    nc.scalar.activation(da[:], deltaT[:, dt, :], AF.Exp,
                                         bias=0.0, scale=acol[:, dt, n:n + 1])
                    dbx = work.tile([P, Lc], F32, tag="dbx")
                    for h in range(2):
                        nc.vector.tensor_mul(dbx[:, h * HLF:(h + 1) * HLF],
                                             uT[:, dt, h * HLF:(h + 1) * HLF],
                                             bb[h][:])
                    hh = work.tile([P, Lc], F32, tag="hh")
                    nc.vector.tensor_tensor_scan(hh[:], da[:], dbx[:], 0.0,
                                                 op0=mult, op1=add)
                    for h in range(2):
                        ch = chp.tile([P, HLF], F32, tag="ch")
                        nc.vector.tensor_mul(ch[:],
                                             hh[:, h * HLF:(h + 1) * HLF],
                                             cc[h][:])
                        ysl = yacc[:, dt, h * HLF:(h + 1) * HLF]
                        if n == 0:
                            nc.vector.tensor_copy(ysl, ch[:])
                        else:
                            nc.vector.tensor_add(ysl, ysl, ch[:])

        # ========== phase 5: transpose y, negate, store ==========
        with (
            tc.tile_pool(name="ps5", bufs=4, space="PSUM") as ps5,
            tc.tile_pool(name="yout", bufs=4) as yout,
        ):
            for i in range(NT):
                yt_ps = ps5.tile([P, DSH], F32, tag="yt")
                for dt in range(NDT):
                    nc.tensor.transpose(yt_ps[:, dt * P:(dt + 1) * P],
                                        yacc[:, dt, i * P:(i + 1) * P],
                                        ident[:])
                yt = yout.tile([P, DSH], F32, tag="yt_sb")
                nc.scalar.copy(yt[:], yt_ps[:])
                nc.sync.dma_start(y_d[i * P:(i + 1) * P, :], yt[:])


def _prep_inputs(hidden_states, W_xproj, W_dt, b_dt, A_log):
    hidden_states = np.asarray(hidden_states, np.float32)
    W_xproj = np.asarray(W_xproj, np.float32)
    W_dt = np.asarray(W_dt, np.float32)
    b_dt = np.asarray(b_dt, np.float32)
    A_log = np.asarray(A_log, np.float32)

    A = -np.exp(A_log)                      # (D, N), negative
    ident = np.eye(P, dtype=np.float32)
    wxT = W_xproj.T                         # (D, E)
    selbc = np.zeros((2 * N, 2 * N * P), np.float32)
    for q in range(2 * N):
        selbc[q, q * P:(q + 1) * P] = 1.0

    in_maps = []
    for core in range(NCORES):
        b, ds = divmod(core, 4)
        sl = slice(ds * DSH, (ds + 1) * DSH)
        perm = np.r_[np.arange(ds * DSH, (ds + 1) * DSH),
                     np.arange(0, ds * DSH),
                     np.arange((ds + 1) * DSH, D)]
        in_maps.append({
            "x": np.ascontiguousarray(hidden_states[b][:, perm]),
            "wxT": np.ascontiguousarray(wxT[perm, :]),
            "wdtT": np.ascontiguousarray(W_dt[sl, :].T),
            "bdt": np.ascontiguousarray(b_dt[sl].reshape(DSH, 1)),
            "acol": np.ascontiguousarray(A[sl, :]),
            "ident": ident,
            "selbc": selbc,
        })
    return in_maps


def kernel(hidden_states, W_xproj, W_dt, b_dt, A_log, _trace=False):
    if "nc" not in _CACHE:
        _CACHE["nc"] = build_nc()
    nc = _CACHE["nc"]
    in_maps = _prep_inputs(hidden_states, W_xproj, W_dt, b_dt, A_log)
    res = run_bass_kernel_spmd(nc, in_maps, core_ids=list(range(NCORES)),
                               trace=_trace)
    _CACHE["last_result"] = res
    out = np.empty((B, L, D), np.float32)
    for core in range(NCORES):
        b, ds = divmod(core, 4)
        out[b, :, ds * DSH:(ds + 1) * DSH] = res.results[core]["y"]
    return out


# revision 7
# speedup vs baseline: 1.1544x; 1.1544x over previous
"""Mamba-1 selective-scan recurrence kernel for Trainium2 (8 NeuronCores).

Problem: B=2, L=2048, D=1024, N=16, R=64 (f32).
  x_dbl = hidden @ W_xproj.T ; dt_low, Bm, Cm = split(x_dbl, [R, N, N])
  delta = softplus(dt_low @ W_dt.T + b_dt)
  h_t   = exp(delta_t*A) * h_{t-1} + (delta_t*x_t) * B_t ;  y_t = sum_n C_t(n) h_t(:,n)

Sharding: core = (batch b in {0,1}) x (channel quarter ds in {0..3}); each core
computes y for its 256 channels of one batch.  No cross-core communication.
The per-core input `x` is hidden[b] with columns permuted so the core's own
channel slice sits at columns [0:256] (the SPMD program is identical on all
cores; the data layout encodes the shard).

Per-core device pipeline (d-on-partitions layout, time on the free axis):
  1. DMA x (2048,1024) -> SBUF, PE-transpose to XT (d,t).
  2. PE: x_dblT (96,2048) = W_xprojT.T @ XT;  split dt_lowT / (B;C) rows.
  3. PE: pre (256,2048) = W_dtT.T @ dt_lowT;
     ACT: deltaT = Ln(Exp(pre + b_dt) + 1) = softplus(pre + b_dt);
     DVE: uT = deltaT * XT[own 256 channels].
  4. outer loop over the two time halves (PSUM capacity), inner n in 0..15:
     PE selector-matmul (fp32r, 1 cyc/col) broadcasts B/C row n across 128
     partitions into PSUM; ACT: dA = Exp(A[:,n] * deltaT);
     DVE: dBx = uT*Bb; h = tensor_tensor_scan(dA, dBx) (chained across
     halves via saved end-state column); ch = h*Cb (fp32r out);
     PE: yacc(PSUM) += I @ ch  (fp32r identity matmul, accumulating).
  5. per half: ACT copy yacc->SBUF, PE-transpose -> (t,d), DMA out.
"""

import sys

for _p in ("/opt/trn_rl_repo",):
    if _p not in sys.path:
        sys.path.insert(0, _p)

import numpy as np

import concourse.bass as bass  # noqa: F401
import concourse.tile as tile
from concourse import bacc, mybir
from concourse.bass_utils import run_bass_kernel_spmd

F32 = mybir.dt.float32
F32R = mybir.dt.float32r

B, L, D, N, R = 2, 2048, 1024, 16, 64
NCORES = 8
DSH = D // 4          # channels per core
P = 128               # partitions
NDT = DSH // P        # d-tiles per core (2)
E = R + 2 * N         # x_dbl feature dim (96)

_CACHE = {}


def build_nc(Lc=L):
    """Build the per-core Bass program (identical across all cores)."""
    nc = bacc.Bacc("TRN2", target_bir_lowering=False, debug=False,
                   num_devices=NCORES)

    x_d = nc.dram_tensor("x", [Lc, D], F32, kind="ExternalInput")
    wxT_d = nc.dram_tensor("wxT", [D, E], F32, kind="ExternalInput")
    wdtT_d = nc.dram_tensor("wdtT", [R, DSH], F32, kind="ExternalInput")
    bdt_d = nc.dram_tensor("bdt", [DSH, 1], F32, kind="ExternalInput")
    acol_d = nc.dram_tensor("acol", [DSH, N], F32, kind="ExternalInput")
    ident_d = nc.dram_tensor("ident", [P, P], F32, kind="ExternalInput")
    identr_d = nc.dram_tensor("identr", [P, P], F32R, kind="ExternalInput")
    selbc_d = nc.dram_tensor("selbc", [2 * N, 2 * N * P], F32R,
                             kind="ExternalInput")
    y_d = nc.dram_tensor("y", [Lc, DSH], F32, kind="ExternalOutput")

    with tile.TileContext(nc) as tc:
        _emit(tc, nc, x_d, wxT_d, wdtT_d, bdt_d, acol_d, ident_d, identr_d,
              selbc_d, y_d, Lc)
    nc.compile()
    return nc


def _emit(tc, nc, x_d, wxT_d, wdtT_d, bdt_d, acol_d, ident_d, identr_d,
          selbc_d, y_d, Lc):
    mult = mybir.AluOpType.mult
    add = mybir.AluOpType.add
    AF = mybir.ActivationFunctionType

    NT = Lc // P          # time chunks of 128
    NJ = D // P           # full-width d-tiles (8)
    NC512 = Lc // 512
    HLF = Lc // 2
    NTH = HLF // P        # time chunks per half

    with (
        tc.tile_pool(name="persist", bufs=1) as persist,
        tc.tile_pool(name="consts", bufs=1) as consts,
    ):
        ident = consts.tile([P, P], F32, tag="ident")
        nc.sync.dma_start(ident[:], ident_d[:])
        identr = consts.tile([P, P], F32R, tag="identr")
        nc.sync.dma_start(identr[:], identr_d[:])
        acol = consts.tile([P, NDT, N], F32, tag="acol")
        bdt = consts.tile([P, NDT], F32, tag="bdt")
        for dt in range(NDT):
            nc.sync.dma_start(acol[:, dt, :], acol_d[dt * P:(dt + 1) * P, :])
            nc.sync.dma_start(bdt[:, dt:dt + 1], bdt_d[dt * P:(dt + 1) * P, :])
        selbc = consts.tile([2 * N, 2 * N, P], F32R, tag="selbc")
        nc.sync.dma_start(selbc[:], selbc_d[:].rearrange(
            "k (q m) -> k q m", q=2 * N))

        # persistent SBUF tensors
        deltaT = persist.tile([P, NDT, Lc], F32, tag="deltaT")
        uT = persist.tile([P, NDT, Lc], F32, tag="uT")
        bc = persist.tile([2 * N, Lc], F32R, tag="bc")  # rows 0:N B, N:2N C
        dtl = persist.tile([R, Lc], F32, tag="dtl")
        hend = persist.tile([P, NDT * N], F32, tag="hend")

        # ========== phase 1-3: X load + transpose, projections ==========
        with (
            tc.tile_pool(name="xt", bufs=1) as xt_pool,
            tc.tile_pool(name="xload", bufs=6) as xload,
            tc.tile_pool(name="ps_t", bufs=4, space="PSUM") as ps_t,
            tc.tile_pool(name="ps_mm", bufs=2, space="PSUM") as ps_mm,
            tc.tile_pool(name="wpool", bufs=1) as wpool,
        ):
            XT = xt_pool.tile([P, NJ, Lc], F32, tag="XT")
            for ig in range(NT // 4):
                xis = []
                for k in range(4):
                    i = ig * 4 + k
                    xi = xload.tile([P, D], F32, tag="xi")
                    nc.sync.dma_start(xi[:], x_d[i * P:(i + 1) * P, :])
                    xis.append(xi)
                for j in range(NJ):
                    pt = ps_t.tile([P, 512], F32, tag="pt")
                    for k in range(4):
                        nc.tensor.transpose(pt[:, k * P:(k + 1) * P],
                                            xis[k][:, j * P:(j + 1) * P],
                                            ident[:])
                    nc.scalar.copy(XT[:, j, ig * 512:(ig + 1) * 512], pt[:])

            wx = wpool.tile([P, NJ, E], F32, tag="wx")
            for j in range(NJ):
                nc.sync.dma_start(wx[:, j, :], wxT_d[j * P:(j + 1) * P, :])
            wdt = wpool.tile([R, DSH], F32, tag="wdt")
            nc.sync.dma_start(wdt[:], wdtT_d[:])

            # x_dblT (E, Lc); B/C rows land at partitions 64:96 of PSUM and
            # are copied (partition-shifted, fp32r-rounded) to bc at base 0.
            for c in range(NC512):
                xdbl_ps = ps_mm.tile([P, 512], F32, tag="mm")
                for j in range(NJ):
                    nc.tensor.matmul(xdbl_ps[0:E, :], wx[:, j, :],
                                     XT[:, j, c * 512:(c + 1) * 512],
                                     start=(j == 0), stop=(j == NJ - 1))
                nc.scalar.copy(dtl[:, c * 512:(c + 1) * 512], xdbl_ps[0:R, :])
                nc.scalar.copy(bc[:, c * 512:(c + 1) * 512], xdbl_ps[R:E, :])

            # deltaT = Ln(Exp(pre + b_dt) + 1) = softplus; uT = deltaT * x
            for dt in range(NDT):
                for c in range(NC512):
                    dp = ps_mm.tile([P, 512], F32, tag="mm")
                    nc.tensor.matmul(dp[:], wdt[:, dt * P:(dt + 1) * P],
                                     dtl[:, c * 512:(c + 1) * 512],
                                     start=True, stop=True)
                    nc.scalar.activation(
                        deltaT[:, dt, c * 512:(c + 1) * 512], dp[:],
                        AF.Exp, bias=bdt[:, dt:dt + 1], scale=1.0)
                for hf in range(2):
                    sl = slice(hf * HLF, (hf + 1) * HLF)
                    nc.scalar.activation(deltaT[:, dt, sl], deltaT[:, dt, sl],
                                         AF.Ln, bias=1.0, scale=1.0)
                    nc.vector.tensor_mul(uT[:, dt, sl], deltaT[:, dt, sl],
                                         XT[:, dt, sl])

        # ========== phase 4+5: per-half recurrence + output ==========
        for hf in range(2):
            h0, h1 = hf * HLF, (hf + 1) * HLF
            with tc.tile_pool(name=f"yps{hf}", bufs=1, space="PSUM") as yps:
                yacc_ps = [yps.tile([P, HLF], F32, name=f"yacc{hf}_{dt}",
                                    tag=f"yacc{dt}")
                           for dt in range(NDT)]
                with (
                    tc.tile_pool(name=f"bcps{hf}", bufs=1,
                                 space="PSUM") as bcps,
                    tc.tile_pool(name=f"work{hf}", bufs=4) as work,
                    tc.tile_pool(name=f"chp{hf}", bufs=3) as chp,
                ):
                    for n in range(N):
                        bbt = bcps.tile([P, HLF], F32, tag="bb")
                        cct = bcps.tile([P, HLF], F32, tag="cc")
                        for q in range(HLF // 512):
                            col = h0 + q * 512
                            qs = slice(q * 512, (q + 1) * 512)
                            nc.tensor.matmul(bbt[:, qs], selbc[:, n, :],
                                             bc[:, col:col + 512],
                                             start=True, stop=True)
                            nc.tensor.matmul(cct[:, qs], selbc[:, N + n, :],
                                             bc[:, col:col + 512],
                                             start=True, stop=True)
                        for dt in range(NDT):
                            da = work.tile([P, HLF], F32, tag="da")
                            nc.scalar.activation(da[:], deltaT[:, dt, h0:h1],
                                                 AF.Exp, bias=0.0,
                                                 scale=acol[:, dt, n:n + 1])
                            dbx = work.tile([P, HLF], F32, tag="dbx")
                            nc.vector.tensor_mul(dbx[:], uT[:, dt, h0:h1],
                                                 bbt[:])
                            hh = work.tile([P, HLF], F32, tag="hh")
                            col = n * NDT + dt
                            init = 0.0 if hf == 0 else hend[:, col:col + 1]
                            nc.vector.tensor_tensor_scan(hh[:], da[:], dbx[:],
                                                         init, op0=mult,
                                                         op1=add)
                            if hf == 0:
                                nc.scalar.copy(hend[:, col:col + 1],
                                               hh[:, HLF - 1:HLF])
                            ch = chp.tile([P, HLF], F32R, tag="ch")
                            nc.vector.tensor_mul(ch[:], hh[:], cct[:])
                            for q in range(HLF // 512):
                                qs = slice(q * 512, (q + 1) * 512)
                                nc.tensor.matmul(yacc_ps[dt][:, qs],
                                                 identr[:], ch[:, qs],
                                                 start=(n == 0),
                                                 stop=(n == N - 1))

                # ---- drain the half: yacc -> SBUF -> transpose -> HBM ----
                with (
                    tc.tile_pool(name=f"psy{hf}", bufs=4, space="PSUM") as psy,
                    tc.tile_pool(name=f"ysb{hf}", bufs=1) as ysbp,
                    tc.tile_pool(name=f"yout{hf}", bufs=4) as yout,
                ):
                    ysb = ysbp.tile([P, NDT, HLF], F32, tag="ysb")
                    for dt in range(NDT):
                        nc.scalar.copy(ysb[:, dt, :], yacc_ps[dt][:])
                    for i in range(NTH):
                        yt_ps = psy.tile([P, DSH], F32, tag="yt")
                        for dt in range(NDT):
                            nc.tensor.transpose(
                                yt_ps[:, dt * P:(dt + 1) * P],
                                ysb[:, dt, i * P:(i + 1) * P], ident[:])
                        yt = yout.tile([P, DSH], F32, tag="yt_sb")
                        nc.scalar.copy(yt[:], yt_ps[:])
                        nc.sync.dma_start(
                            y_d[h0 + i * P:h0 + (i + 1) * P, :], yt[:])


def _prep_inputs(hidden_states, W_xproj, W_dt, b_dt, A_log):
    hidden_states = np.asarray(hidden_states, np.float32)
    W_xproj = np.asarray(W_xproj, np.float32)
    W_dt = np.asarray(W_dt, np.float32)
    b_dt = np.asarray(b_dt, np.float32)
    A_log = np.asarray(A_log, np.float32)

    A = -np.exp(A_log)                      # (D, N), negative
    ident = np.eye(P, dtype=np.float32)
    wxT = W_xproj.T                         # (D, E)
    selbc = np.zeros((2 * N, 2 * N * P), np.float32)
    for q in range(2 * N):
        selbc[q, q * P:(q + 1) * P] = 1.0

    in_maps = []
    for core in range(NCORES):
        b, ds = divmod(core, 4)
        sl = slice(ds * DSH, (ds + 1) * DSH)
        perm = np.r_[np.arange(ds * DSH, (ds + 1) * DSH),
                     np.arange(0, ds * DSH),
                     np.arange((ds + 1) * DSH, D)]
        in_maps.append({
            "x": np.ascontiguousarray(hidden_states[b][:, perm]),
            "wxT": np.ascontiguousarray(wxT[perm, :]),
            "wdtT": np.ascontiguousarray(W_dt[sl, :].T),
            "bdt": np.ascontiguousarray(b_dt[sl].reshape(DSH, 1)),
            "acol": np.ascontiguousarray(A[sl, :]),
            "ident": ident,
            "identr": ident,
            "selbc": selbc,
        })
    return in_maps


def kernel(hidden_states, W_xproj, W_dt, b_dt, A_log, _trace=False):
    if "nc" not in _CACHE:
        _CACHE["nc"] = build_nc()
    nc = _CACHE["nc"]
    in_maps = _prep_inputs(hidden_states, W_xproj, W_dt, b_dt, A_log)
    res = run_bass_kernel_spmd(nc, in_maps, core_ids=list(range(NCORES)),
                               trace=_trace)
    _CACHE["last_result"] = res
    out = np.empty((B, L, D), np.float32)
    for core in range(NCORES):
        b, ds = divmod(core, 4)
        out[b, :, ds * DSH:(ds + 1) * DSH] = res.results[core]["y"]
    return out
